# revision 72
# baseline (speedup 1.0000x reference)
"""Conv2d 3x3 (stride 1, pad 1) NCHW kernel for 8 Trainium2 NeuronCores.

Problem: x (32,128,56,56) f32, weight (256,128,3,3), bias (256,)
         -> out (32,256,56,56), same-padding conv + bias.

Strategy:
  - Data parallel: 4 images per core across 8 cores (batch shard).
  - fp8 DoubleRow implicit GEMM.  Each operand is split hi/lo into two
    fp8e4m3 planes (x ~ x_hi + x_lo, w ~ w_hi + w_lo).  A DoubleRow
    matmul contracts TWO independent 128-deep products per output
    column at 0.5 cycles/row (4 products/cycle -- the TRN2 PE maximum).
    Of the 27 useful tap-products per output tile we keep 24, packed
    into exactly 12 DR matmuls per group:
      * 4 main pairs   (w_hi@t,  w_hi@t+1) x (x_hi@t, x_hi@t+1), t=0..7
      * 1 combo        (w_hi@8,  w_hi@8')  x (x_lo@8, x_hi@8)
        (the tap-9 slot of the hi plane holds a copy of w_hi@8)
      * 7 crosses      (w_hi@t,  w_lo@t)   x (x_lo@t, x_hi@t), t=0..6
    Dropping the three remaining corrections (x-cross 7, w-cross 7/8)
    costs ~0.9e-2 rel err each in quadrature: measured 1.56e-2, inside
    the 2e-2 gate.  (11 DR = 5 drops would be 2.01e-2 -- just over.)
  - Layout: the padded image (58x58) is stored row-contiguous per
    plane; the moving operand is a 4D pattern [c, plane-pair, row(58),
    col(56)] so the PSUM tile is junk-free (ap_size 448, min cycles).
  - The cost model serializes HWDGE descriptor-gen (~625ns/DMA, all
    rings) and all transfers on one 360B/ns bus, with +900ns sem prop
    per DMA.  Startup DMAs ride the SP ring in deadline order (w-hi
    both oc, x heads, x rests, w-lo per oc); bias generates on the
    parallel Pool SWDGE path.  Image 0's first groups are emitted
    mains-first so the PE always has hi-plane work while lo pieces
    land.  Warm-up matmuls on a memset tile cover the p-state ramp
    (full clock needs 3us of PE busy before the first real decode).
  - Output is stored bf16 (halves store-bus traffic; upcast on host;
    +0.1% rel err in quadrature).  Bias is fused into the PSUM->SBUF
    eviction (ACT/DVE alternating).  The final group runs as two 4-row
    halves evicted on ACT as each stops, into one tile with one store,
    so only ~half an eviction plus one store chain trails the last
    matmul.
"""

import numpy as np

N_CORES = 8
N, C, H, W = 32, 128, 56, 56
O = 256
PAD = 1
HP = H + 2 * PAD  # 58
ROWB = W + 2 * PAD  # 58 elements per stored row
PLANE = 3368  # 58*58 = 3364 rounded up (pad matmul reads 2 past the end)
NPC = N // N_CORES  # images per core = 4
RPC = 8  # output rows per chunk
N_CHUNKS = H // RPC  # 7
OC_TILES = O // 128  # 2
NTAPS = 9
N_MM = 12  # DR matmuls per group (24 fp8 product slots)
# tap t=(kh,kw) shifts the flat window by s(t) = kh*58 + kw
TAP_S = [(t // 3) * ROWB + (t % 3) for t in range(NTAPS)]
# weight tile layout per partition c: [hi-oc0(9t), hi-oc1(9t), lo-oc0(7t),
# lo-oc1(7t)] x m -- the combo matmul re-reads w_hi@8 via a 0-stride plane
# dim, so no tap-9 copy slot is needed and the lo planes carry only the
# 7 cross-corrected taps.
WHI = 9 * 128  # 1152 per hi plane
WLO = 7 * 128  # 896 per lo plane
WTOT = 2 * WHI + 2 * WLO  # 4096 per partition
WTAP = 128

_CACHE = {}
LAST_RESULTS = None


def _build():
    import concourse.bass as bass
    import concourse.bacc as bacc
    import concourse.mybir as mybir
    import concourse.tile as tile
    from concourse.ap import AP

    f32 = mybir.dt.float32
    bf16 = mybir.dt.bfloat16
    f8 = mybir.dt.float8e4
    DR = mybir.MatmulPerfMode.DoubleRow

    nc = bacc.Bacc(
        "TRN2", target_bir_lowering=False, debug=False, num_devices=N_CORES
    )
    xp_d = nc.dram_tensor("xp", (NPC, C, 2, PLANE), f8, kind="ExternalInput")
    w_d = nc.dram_tensor("w2", (C, WTOT), f8, kind="ExternalInput")
    b_d = nc.dram_tensor("b2", (128, OC_TILES), f32, kind="ExternalInput")
    # bf16 output (upcast to f32 on the host): halves store DMA traffic.
    out_d = nc.dram_tensor("out", (NPC, O, H, W), bf16, kind="ExternalOutput")

    with tile.TileContext(nc) as tc:
        with (
            tc.tile_pool(name="w", bufs=1) as wpool,
            tc.tile_pool(name="x", bufs=2) as xpool,
            tc.tile_pool(name="ps", bufs=7, space=bass.MemorySpace.PSUM) as pspool,
            tc.tile_pool(name="psw", bufs=1, space=bass.MemorySpace.PSUM) as pspool_w,
            tc.tile_pool(name="o", bufs=8) as opool,
        ):
            w_t = wpool.tile([C, WTOT], f8)
            b_t = wpool.tile([128, OC_TILES], f32)
            # PE p-state warmup: matmuls decode at half rate until the PE
            # has been busy 3us.  Dummy DoubleRow matmuls on a memset tile
            # start that clock at ~1.05us and bridge until the first real
            # matmul's data lands (~4.3us), so the real stream decodes at
            # full clock from its first instruction.
            warm_t = wpool.tile([128, 256], f8)
            warm_ps = pspool_w.tile([128, 128], f32)
            nc.gpsimd.memset(warm_t[:], 0)
            warm_lhs = AP(
                warm_t.tensor, warm_t.offset, [[256, 128], [128, 2], [1, 128]]
            )
            warm_rhs = AP(
                warm_t.tensor, warm_t.offset, [[256, 128], [128, 2], [1, 128]]
            )
            warm_out = AP(warm_ps.tensor, warm_ps.offset, [[128, 128], [1, 128]])
            for _ in range(58):
                nc.tensor.matmul(
                    warm_out,
                    warm_lhs,
                    warm_rhs,
                    start=True,
                    stop=True,
                    perf_mode=DR,
                )
            # Startup-critical DMAs all ride the SP ring in deadline order
            # (HWDGE descriptor-gen is one shared serial device, and all
            # transfers serialize on one DMA bus, so order is everything).
            # The hi weight planes of BOTH oc tiles come first: they gate
            # every group's main-pair matmuls.
            nc.sync.dma_start(w_t[:, 0 : 2 * WHI], w_d[:, 0 : 2 * WHI])
            # bias generates on the parallel Pool SWDGE path; the time floor
            # keeps its bus slot out of the startup-critical window.
            with tc.tile_wait_until(0.002):
                nc.gpsimd.dma_start(b_t[:], b_d[:])

            def wap(oc, off, d1):
                # off is relative to the oc's hi plane; cross matmuls reach
                # the oc's lo plane at d1 = 2*WHI + oc*WLO - oc*WHI ...
                # passed explicitly by the callers.
                return AP(
                    w_t.tensor,
                    w_t.offset + oc * WHI + off,
                    [[WTOT, 128], [d1, 2], [1, 128]],
                )

            def xap4(x_t, base, off, d1, rows):
                # 4D moving operand: [c, plane-pair, out-row, out-col] walks
                # only the 56 useful columns of each output row, so the PSUM
                # tile is junk-free and ap_size drops 462 -> rows*56.
                return AP(
                    x_t.tensor,
                    x_t.offset + base + off,
                    [[2 * PLANE, 128], [d1, 2], [ROWB, rows], [1, W]],
                )

            def emit_mains(x_t, ps_ap, base, oc, rows=RPC):
                # 4 main-pair matmuls (w_hi@t, w_hi@t+1) x (x_hi@t, x_hi@t+1)
                # plus the combo (w_hi@8, w_hi@8 copy in tap-9 slot) x
                # (x_lo@8, x_hi@8) = main tap 8 + its x-side correction.
                # Needs only the hi weight planes + the x heads.
                for i, t in enumerate((0, 2, 4, 6)):
                    nc.tensor.matmul(
                        ps_ap,
                        wap(oc, t * WTAP, WTAP),
                        xap4(x_t, base, PLANE + TAP_S[t], TAP_S[t + 1] - TAP_S[t], rows),
                        start=(i == 0),
                        stop=False,
                        perf_mode=DR,
                    )
                nc.tensor.matmul(
                    ps_ap,
                    wap(oc, 8 * WTAP, 0),
                    xap4(x_t, base, TAP_S[8], PLANE, rows),
                    start=False,
                    stop=False,
                    perf_mode=DR,
                )

            def emit_crosses(x_t, ps_ap, base, oc, rows=RPC):
                # 7 cross matmuls: (w_hi@t, w_lo@t) x (x_lo@t, x_hi@t).
                # Taps 7 and 8 lose their w-side correction and tap 7 its
                # x-side one: rel err ~1.5e-2, inside the 2e-2 gate.
                d1 = 2 * WHI + oc * WLO - oc * WHI - 0 * WTAP
                for t in range(7):
                    nc.tensor.matmul(
                        ps_ap,
                        wap(oc, t * WTAP, d1),
                        xap4(x_t, base, TAP_S[t], PLANE, rows),
                        start=False,
                        stop=(t == 6),
                        perf_mode=DR,
                    )

            def group(x_t, ps_ap, base, oc, rows=RPC):
                emit_mains(x_t, ps_ap, base, oc, rows)
                emit_crosses(x_t, ps_ap, base, oc, rows)

            HEAD = 20 * ROWB  # rows 0-19
            for idx in range(NPC):
                x_t = xpool.tile([C, 2, PLANE], f8)
                if idx == 0:
                    # image 0 heads: hi head gates chunk-0/1 mains, lo head
                    # their crosses; w-oc1 slots in before the rests.
                    # per-plane pieces keep each DMA's written range flat and
                    # contiguous, so chunk reads only wait for their pieces.
                    nc.sync.dma_start(x_t[:, 1, 0:HEAD], xp_d[0, :, 1, 0:HEAD])
                    nc.sync.dma_start(x_t[:, 1, HEAD:PLANE], xp_d[0, :, 1, HEAD:PLANE])
                    nc.sync.dma_start(x_t[:, 0, 0:HEAD], xp_d[0, :, 0, 0:HEAD])
                    nc.sync.dma_start(x_t[:, 0, HEAD:PLANE], xp_d[0, :, 0, HEAD:PLANE])
                    nc.sync.dma_start(
                        w_t[:, 2 * WHI : 2 * WHI + WLO],
                        w_d[:, 2 * WHI : 2 * WHI + WLO],
                    )
                    nc.sync.dma_start(
                        w_t[:, 2 * WHI + WLO : WTOT],
                        w_d[:, 2 * WHI + WLO : WTOT],
                    )
                else:
                    # later images ride the SP ring behind image 0 and the
                    # early stores; the manual wait keeps their 2.4us
                    # transfers from hoisting ahead of startup-critical DMAs.
                    with tc.tile_wait_until(0.005 * idx):
                        nc.sync.dma_start(x_t[:], xp_d[idx])
                def evict_store(ps, ch, oc, on_act):
                    bias_ap = b_t[:, oc : oc + 1]
                    out_ap = out_d[
                        idx, oc * 128 : (oc + 1) * 128, ch * RPC : (ch + 1) * RPC, :
                    ]
                    o_t = opool.tile([128, RPC, W], bf16)
                    if on_act:
                        nc.scalar.add(o_t[:], ps[:], bias_ap)
                    else:
                        nc.vector.tensor_scalar_add(o_t[:], ps[:], bias_ap)
                    nc.sync.dma_start(out_ap, o_t[:])

                if idx == 0:
                    # the lo weight planes land ~1.5us after the hi planes:
                    # run the mains of the first four groups back-to-back
                    # (they need only hi planes + x heads), then their
                    # crosses once the lo pieces have arrived.
                    head6 = [(0, 0), (1, 0), (0, 1), (1, 1)]
                    ps6 = []
                    for ch, oc in head6:
                        ps = pspool.tile([128, RPC, W], f32)
                        ps6.append(ps)
                        emit_mains(x_t, ps[:], ch * RPC * ROWB, oc)
                    for (ch, oc), ps in zip(head6, ps6):
                        emit_crosses(x_t, ps[:], ch * RPC * ROWB, oc)
                        evict_store(ps, ch, oc, on_act=(oc == 0))
                    sched = [
                        (ch, oc) for ch in range(2, N_CHUNKS) for oc in range(OC_TILES)
                    ]
                elif idx == NPC - 1:
                    # hoist (6,0) so only the final (6,1) group trails,
                    # keeping the end-of-kernel store convoy shallow.
                    sched = [(6, 0)] + [
                        (ch, oc) for ch in range(N_CHUNKS - 1) for oc in range(OC_TILES)
                    ] + [(6, 1)]
                else:
                    sched = [
                        (ch, oc) for ch in range(N_CHUNKS) for oc in range(OC_TILES)
                    ]
                for gi, (ch, oc) in enumerate(sched):
                    base = ch * RPC * ROWB
                    is_last = (
                        idx == NPC - 1 and ch == N_CHUNKS - 1 and oc == OC_TILES - 1
                    )
                    if is_last:
                        # final chunk as four 2-row accumulation groups:
                        # q1/q2 evict on ACT into tileA, q3/q4 on DVE into
                        # tileB (separate tiles -- no WAW serialization), so
                        # the evictions pipeline with the last matmuls on
                        # two engines and two stores overlap the tail.
                        bias_ap = b_t[:, oc : oc + 1]
                        out_ap = out_d[
                            idx, oc * 128 : (oc + 1) * 128,
                            ch * RPC : (ch + 1) * RPC, :,
                        ]
                        h = 2
                        o_tA = opool.tile([128, 4, W], bf16, tag="oA")
                        o_tB = opool.tile([128, 4, W], bf16, tag="oB")
                        for q in range(RPC // h):
                            psQ = pspool.tile([128, h, W], f32, tag="ps")
                            group(x_t, psQ[:], base + q * h * ROWB, oc, rows=h)
                            dst = o_tA if q < 2 else o_tB
                            sl = dst[:, (q % 2) * h : (q % 2 + 1) * h]
                            if q < 2:
                                nc.scalar.add(sl, psQ[:], bias_ap)
                            else:
                                nc.vector.tensor_scalar_add(sl, psQ[:], bias_ap)
                        nc.sync.dma_start(out_ap[:, 0:4, :], o_tA[:])
                        nc.sync.dma_start(out_ap[:, 4:RPC, :], o_tB[:])
                        continue
                    ps = pspool.tile([128, RPC, W], f32)
                    group(x_t, ps[:], base, oc)
                    # on the last image keep ACT clear ahead of the final
                    # group's two ACT evictions
                    evict_store(ps, ch, oc, on_act=(gi % 2 == (idx == NPC - 1)))
    nc.compile()
    return nc


def kernel(x, weight, bias):
    global LAST_RESULTS
    import ml_dtypes
    from concourse.bass_utils import run_bass_kernel_spmd

    f8 = ml_dtypes.float8_e4m3
    x = np.asarray(x, dtype=np.float32)
    weight = np.asarray(weight, dtype=np.float32)
    bias = np.asarray(bias, dtype=np.float32)

    # padded row-contiguous image planes: [N, C, 2(lo,hi), PLANE] fp8
    xpad = np.zeros((N, C, HP, ROWB), np.float32)
    xpad[:, :, PAD : PAD + H, PAD : PAD + W] = x
    xpad = xpad.reshape(N, C, HP * ROWB)
    x_hi = xpad.astype(f8)
    x_lo = (xpad - x_hi.astype(np.float32)).astype(f8)
    xp = np.zeros((N, C, 2, PLANE), f8)
    xp[:, :, 0, : HP * ROWB] = x_lo
    xp[:, :, 1, : HP * ROWB] = x_hi

    # weights: [C, oc_tile, plane(hi,lo), tap(10), m(128)] fp8, tap 9 = 0
    # wt[o, c, t] = weight[o, c, kh, kw]
    wt = weight.reshape(O, C, 9)
    w_hi = wt.astype(f8)
    w_lo = (wt - w_hi.astype(np.float32)).astype(f8)
    # packed layout per c: [hi-oc0 9 taps | hi-oc1 9 taps | lo-oc0 7 taps |
    # lo-oc1 7 taps] x m.  The combo matmul re-reads w_hi@8 with a 0-stride
    # plane dim, so no copy slot; lo taps 7/8 are the dropped corrections.
    whi_t = w_hi.reshape(OC_TILES, 128, C, 9).transpose(2, 0, 3, 1)
    wlo_t = w_lo.reshape(OC_TILES, 128, C, 9).transpose(2, 0, 3, 1)
    w2 = np.zeros((C, WTOT), f8)
    w2[:, 0 : 2 * WHI] = whi_t.reshape(C, 2 * WHI)
    w2[:, 2 * WHI : 2 * WHI + WLO] = wlo_t[:, 0, :7, :].reshape(C, WLO)
    w2[:, 2 * WHI + WLO : WTOT] = wlo_t[:, 1, :7, :].reshape(C, WLO)

    b2 = np.ascontiguousarray(bias.reshape(OC_TILES, 128).T)

    if "nc" not in _CACHE:
        _CACHE["nc"] = _build()
    nc = _CACHE["nc"]

    in_maps = [
        {"xp": xp[i * NPC : (i + 1) * NPC], "w2": w2, "b2": b2}
        for i in range(N_CORES)
    ]
    res = run_bass_kernel_spmd(nc, in_maps, core_ids=list(range(N_CORES)))
    LAST_RESULTS = res
    return np.concatenate(
        [np.asarray(r["out"]).astype(np.float32) for r in res.results], axis=0
    )



# revision 73
# speedup vs baseline: 1.0061x; 1.0061x over previous
"""Conv2d 3x3 (stride 1, pad 1) NCHW kernel for 8 Trainium2 NeuronCores.

Problem: x (32,128,56,56) f32, weight (256,128,3,3), bias (256,)
         -> out (32,256,56,56), same-padding conv + bias.

Strategy:
  - Data parallel: 4 images per core across 8 cores (batch shard).
  - fp8 DoubleRow implicit GEMM.  Each operand is split hi/lo into two
    fp8e4m3 planes (x ~ x_hi + x_lo, w ~ w_hi + w_lo).  A DoubleRow
    matmul contracts TWO independent 128-deep products per output
    column at 0.5 cycles/row (4 products/cycle -- the TRN2 PE maximum).
    Of the 27 useful tap-products per output tile we keep 24, packed
    into exactly 12 DR matmuls per group:
      * 4 main pairs   (w_hi@t,  w_hi@t+1) x (x_hi@t, x_hi@t+1), t=0..7
      * 1 combo        (w_hi@8,  w_hi@8')  x (x_lo@8, x_hi@8)
        (the tap-9 slot of the hi plane holds a copy of w_hi@8)
      * 7 crosses      (w_hi@t,  w_lo@t)   x (x_lo@t, x_hi@t), t=0..6
    Dropping the three remaining corrections (x-cross 7, w-cross 7/8)
    costs ~0.9e-2 rel err each in quadrature: measured 1.56e-2, inside
    the 2e-2 gate.  (11 DR = 5 drops would be 2.01e-2 -- just over.)
  - Layout: the padded image (58x58) is stored row-contiguous per
    plane; the moving operand is a 4D pattern [c, plane-pair, row(58),
    col(56)] so the PSUM tile is junk-free (ap_size 448, min cycles).
  - The cost model serializes HWDGE descriptor-gen (~625ns/DMA, all
    rings) and all transfers on one 360B/ns bus, with +900ns sem prop
    per DMA.  Startup DMAs ride the SP ring in deadline order (w-hi
    both oc, x heads, x rests, w-lo per oc); bias generates on the
    parallel Pool SWDGE path.  Image 0's first groups are emitted
    mains-first so the PE always has hi-plane work while lo pieces
    land.  Warm-up matmuls on a memset tile cover the p-state ramp
    (full clock needs 3us of PE busy before the first real decode).
  - Output is stored bf16 (halves store-bus traffic; upcast on host;
    +0.1% rel err in quadrature).  Bias is fused into the PSUM->SBUF
    eviction (ACT/DVE alternating).  The final group runs as two 4-row
    halves evicted on ACT as each stops, into one tile with one store,
    so only ~half an eviction plus one store chain trails the last
    matmul.
"""

import numpy as np

N_CORES = 8
N, C, H, W = 32, 128, 56, 56
O = 256
PAD = 1
HP = H + 2 * PAD  # 58
ROWB = W + 2 * PAD  # 58 elements per stored row
PLANE = 3368  # 58*58 = 3364 rounded up (pad matmul reads 2 past the end)
NPC = N // N_CORES  # images per core = 4
RPC = 8  # output rows per chunk
N_CHUNKS = H // RPC  # 7
OC_TILES = O // 128  # 2
NTAPS = 9
N_MM = 12  # DR matmuls per group (24 fp8 product slots)
# tap t=(kh,kw) shifts the flat window by s(t) = kh*58 + kw
TAP_S = [(t // 3) * ROWB + (t % 3) for t in range(NTAPS)]
# weight tile layout per partition c: [hi-oc0(9t), hi-oc1(9t), lo-oc0(7t),
# lo-oc1(7t)] x m -- the combo matmul re-reads w_hi@8 via a 0-stride plane
# dim, so no tap-9 copy slot is needed and the lo planes carry only the
# 7 cross-corrected taps.
WHI = 9 * 128  # 1152 per hi plane
WLO = 7 * 128  # 896 per lo plane
WTOT = 2 * WHI + 2 * WLO  # 4096 per partition
WTAP = 128

_CACHE = {}
LAST_RESULTS = None


def _build():
    import concourse.bass as bass
    import concourse.bacc as bacc
    import concourse.mybir as mybir
    import concourse.tile as tile
    from concourse.ap import AP

    f32 = mybir.dt.float32
    bf16 = mybir.dt.bfloat16
    f8 = mybir.dt.float8e4
    DR = mybir.MatmulPerfMode.DoubleRow

    nc = bacc.Bacc(
        "TRN2", target_bir_lowering=False, debug=False, num_devices=N_CORES
    )
    xp_d = nc.dram_tensor("xp", (NPC, C, 2, PLANE), f8, kind="ExternalInput")
    w_d = nc.dram_tensor("w2", (C, WTOT), f8, kind="ExternalInput")
    b_d = nc.dram_tensor("b2", (128, OC_TILES), f32, kind="ExternalInput")
    # bf16 output (upcast to f32 on the host): halves store DMA traffic.
    out_d = nc.dram_tensor("out", (NPC, O, H, W), bf16, kind="ExternalOutput")

    with tile.TileContext(nc) as tc:
        with (
            tc.tile_pool(name="w", bufs=1) as wpool,
            tc.tile_pool(name="x", bufs=2) as xpool,
            tc.tile_pool(name="ps", bufs=7, space=bass.MemorySpace.PSUM) as pspool,
            tc.tile_pool(name="psw", bufs=1, space=bass.MemorySpace.PSUM) as pspool_w,
            tc.tile_pool(name="o", bufs=8) as opool,
        ):
            w_t = wpool.tile([C, WTOT], f8)
            b_t = wpool.tile([128, OC_TILES], f32)
            # PE p-state warmup: matmuls decode at half rate until the PE
            # has been busy 3us.  Dummy DoubleRow matmuls on a memset tile
            # start that clock at ~1.05us and bridge until the first real
            # matmul's data lands (~4.3us), so the real stream decodes at
            # full clock from its first instruction.
            warm_t = wpool.tile([128, 256], f8)
            warm_ps = pspool_w.tile([128, 128], f32)
            nc.gpsimd.memset(warm_t[:], 0)
            warm_lhs = AP(
                warm_t.tensor, warm_t.offset, [[256, 128], [128, 2], [1, 128]]
            )
            warm_rhs = AP(
                warm_t.tensor, warm_t.offset, [[256, 128], [128, 2], [1, 128]]
            )
            warm_out = AP(warm_ps.tensor, warm_ps.offset, [[128, 128], [1, 128]])
            for _ in range(58):
                nc.tensor.matmul(
                    warm_out,
                    warm_lhs,
                    warm_rhs,
                    start=True,
                    stop=True,
                    perf_mode=DR,
                )
            # Startup-critical DMAs all ride the SP ring in deadline order
            # (HWDGE descriptor-gen is one shared serial device, and all
            # transfers serialize on one DMA bus, so order is everything).
            # The hi weight planes of BOTH oc tiles come first: they gate
            # every group's main-pair matmuls.
            nc.sync.dma_start(w_t[:, 0 : 2 * WHI], w_d[:, 0 : 2 * WHI])
            # bias generates on the parallel Pool SWDGE path; the time floor
            # keeps its bus slot out of the startup-critical window.
            with tc.tile_wait_until(0.002):
                nc.gpsimd.dma_start(b_t[:], b_d[:])

            def wap(oc, off, d1):
                # off is relative to the oc's hi plane; cross matmuls reach
                # the oc's lo plane at d1 = 2*WHI + oc*WLO - oc*WHI ...
                # passed explicitly by the callers.
                return AP(
                    w_t.tensor,
                    w_t.offset + oc * WHI + off,
                    [[WTOT, 128], [d1, 2], [1, 128]],
                )

            def xap4(x_t, base, off, d1, rows):
                # 4D moving operand: [c, plane-pair, out-row, out-col] walks
                # only the 56 useful columns of each output row, so the PSUM
                # tile is junk-free and ap_size drops 462 -> rows*56.
                return AP(
                    x_t.tensor,
                    x_t.offset + base + off,
                    [[2 * PLANE, 128], [d1, 2], [ROWB, rows], [1, W]],
                )

            def emit_mains(x_t, ps_ap, base, oc, rows=RPC):
                # 4 main-pair matmuls (w_hi@t, w_hi@t+1) x (x_hi@t, x_hi@t+1)
                # plus the combo (w_hi@8, w_hi@8 copy in tap-9 slot) x
                # (x_lo@8, x_hi@8) = main tap 8 + its x-side correction.
                # Needs only the hi weight planes + the x heads.
                for i, t in enumerate((0, 2, 4, 6)):
                    nc.tensor.matmul(
                        ps_ap,
                        wap(oc, t * WTAP, WTAP),
                        xap4(x_t, base, PLANE + TAP_S[t], TAP_S[t + 1] - TAP_S[t], rows),
                        start=(i == 0),
                        stop=False,
                        perf_mode=DR,
                    )
                nc.tensor.matmul(
                    ps_ap,
                    wap(oc, 8 * WTAP, 0),
                    xap4(x_t, base, TAP_S[8], PLANE, rows),
                    start=False,
                    stop=False,
                    perf_mode=DR,
                )

            def emit_crosses(x_t, ps_ap, base, oc, rows=RPC):
                # 7 cross matmuls: (w_hi@t, w_lo@t) x (x_lo@t, x_hi@t).
                # Taps 7 and 8 lose their w-side correction and tap 7 its
                # x-side one: rel err ~1.5e-2, inside the 2e-2 gate.
                d1 = 2 * WHI + oc * WLO - oc * WHI - 0 * WTAP
                for t in range(7):
                    nc.tensor.matmul(
                        ps_ap,
                        wap(oc, t * WTAP, d1),
                        xap4(x_t, base, TAP_S[t], PLANE, rows),
                        start=False,
                        stop=(t == 6),
                        perf_mode=DR,
                    )

            def group(x_t, ps_ap, base, oc, rows=RPC):
                emit_mains(x_t, ps_ap, base, oc, rows)
                emit_crosses(x_t, ps_ap, base, oc, rows)

            HEAD = 20 * ROWB  # rows 0-19
            for idx in range(NPC):
                x_t = xpool.tile([C, 2, PLANE], f8)
                if idx == 0:
                    # image 0 heads: hi head gates chunk-0/1 mains, lo head
                    # their crosses; w-oc1 slots in before the rests.
                    # per-plane pieces keep each DMA's written range flat and
                    # contiguous, so chunk reads only wait for their pieces.
                    nc.sync.dma_start(x_t[:, 1, 0:HEAD], xp_d[0, :, 1, 0:HEAD])
                    nc.sync.dma_start(x_t[:, 1, HEAD:PLANE], xp_d[0, :, 1, HEAD:PLANE])
                    nc.sync.dma_start(x_t[:, 0, 0:HEAD], xp_d[0, :, 0, 0:HEAD])
                    nc.sync.dma_start(x_t[:, 0, HEAD:PLANE], xp_d[0, :, 0, HEAD:PLANE])
                    nc.sync.dma_start(
                        w_t[:, 2 * WHI : 2 * WHI + WLO],
                        w_d[:, 2 * WHI : 2 * WHI + WLO],
                    )
                    nc.sync.dma_start(
                        w_t[:, 2 * WHI + WLO : WTOT],
                        w_d[:, 2 * WHI + WLO : WTOT],
                    )
                else:
                    # later images ride the SP ring behind image 0 and the
                    # early stores; the manual wait keeps their 2.4us
                    # transfers from hoisting ahead of startup-critical DMAs.
                    with tc.tile_wait_until(0.005 * idx):
                        nc.sync.dma_start(x_t[:], xp_d[idx])
                def evict_store(ps, ch, oc, on_act):
                    bias_ap = b_t[:, oc : oc + 1]
                    out_ap = out_d[
                        idx, oc * 128 : (oc + 1) * 128, ch * RPC : (ch + 1) * RPC, :
                    ]
                    o_t = opool.tile([128, RPC, W], bf16)
                    if on_act:
                        nc.scalar.add(o_t[:], ps[:], bias_ap)
                    else:
                        nc.vector.tensor_scalar_add(o_t[:], ps[:], bias_ap)
                    nc.sync.dma_start(out_ap, o_t[:])

                if idx == 0:
                    # the lo weight planes land ~1.5us after the hi planes:
                    # run the mains of the first four groups back-to-back
                    # (they need only hi planes + x heads), then their
                    # crosses once the lo pieces have arrived.
                    head6 = [(0, 0), (1, 0), (0, 1), (1, 1)]
                    ps6 = []
                    for ch, oc in head6:
                        ps = pspool.tile([128, RPC, W], f32)
                        ps6.append(ps)
                        emit_mains(x_t, ps[:], ch * RPC * ROWB, oc)
                    for (ch, oc), ps in zip(head6, ps6):
                        emit_crosses(x_t, ps[:], ch * RPC * ROWB, oc)
                        evict_store(ps, ch, oc, on_act=(oc == 0))
                    sched = [
                        (ch, oc) for ch in range(2, N_CHUNKS) for oc in range(OC_TILES)
                    ]
                elif idx == NPC - 1:
                    # hoist (6,0) so only the final (6,1) group trails,
                    # keeping the end-of-kernel store convoy shallow.
                    sched = [(6, 0)] + [
                        (ch, oc) for ch in range(N_CHUNKS - 1) for oc in range(OC_TILES)
                    ] + [(6, 1)]
                else:
                    sched = [
                        (ch, oc) for ch in range(N_CHUNKS) for oc in range(OC_TILES)
                    ]
                for gi, (ch, oc) in enumerate(sched):
                    base = ch * RPC * ROWB
                    is_last = (
                        idx == NPC - 1 and ch == N_CHUNKS - 1 and oc == OC_TILES - 1
                    )
                    if is_last:
                        # final chunk as four 2-row accumulation groups,
                        # each evicted on ACT the moment it stops, into one
                        # tile with one store: only a 2-row evict plus the
                        # store chain trails the last matmul.
                        bias_ap = b_t[:, oc : oc + 1]
                        out_ap = out_d[
                            idx, oc * 128 : (oc + 1) * 128,
                            ch * RPC : (ch + 1) * RPC, :,
                        ]
                        o_t = opool.tile([128, RPC, W], bf16)
                        r0 = 0
                        for rows in (3, 3, 2):
                            psQ = pspool.tile([128, rows, W], f32, tag="ps")
                            group(x_t, psQ[:], base + r0 * ROWB, oc, rows=rows)
                            nc.scalar.add(o_t[:, r0 : r0 + rows], psQ[:], bias_ap)
                            r0 += rows
                        nc.sync.dma_start(out_ap, o_t[:])
                        continue
                    ps = pspool.tile([128, RPC, W], f32)
                    group(x_t, ps[:], base, oc)
                    # on the last image keep ACT clear ahead of the final
                    # group's two ACT evictions
                    evict_store(ps, ch, oc, on_act=(gi % 2 == (idx == NPC - 1)))
    nc.compile()
    return nc


def kernel(x, weight, bias):
    global LAST_RESULTS
    import ml_dtypes
    from concourse.bass_utils import run_bass_kernel_spmd

    f8 = ml_dtypes.float8_e4m3
    x = np.asarray(x, dtype=np.float32)
    weight = np.asarray(weight, dtype=np.float32)
    bias = np.asarray(bias, dtype=np.float32)

    # padded row-contiguous image planes: [N, C, 2(lo,hi), PLANE] fp8
    xpad = np.zeros((N, C, HP, ROWB), np.float32)
    xpad[:, :, PAD : PAD + H, PAD : PAD + W] = x
    xpad = xpad.reshape(N, C, HP * ROWB)
    x_hi = xpad.astype(f8)
    x_lo = (xpad - x_hi.astype(np.float32)).astype(f8)
    xp = np.zeros((N, C, 2, PLANE), f8)
    xp[:, :, 0, : HP * ROWB] = x_lo
    xp[:, :, 1, : HP * ROWB] = x_hi

    # weights: [C, oc_tile, plane(hi,lo), tap(10), m(128)] fp8, tap 9 = 0
    # wt[o, c, t] = weight[o, c, kh, kw]
    wt = weight.reshape(O, C, 9)
    w_hi = wt.astype(f8)
    w_lo = (wt - w_hi.astype(np.float32)).astype(f8)
    # packed layout per c: [hi-oc0 9 taps | hi-oc1 9 taps | lo-oc0 7 taps |
    # lo-oc1 7 taps] x m.  The combo matmul re-reads w_hi@8 with a 0-stride
    # plane dim, so no copy slot; lo taps 7/8 are the dropped corrections.
    whi_t = w_hi.reshape(OC_TILES, 128, C, 9).transpose(2, 0, 3, 1)
    wlo_t = w_lo.reshape(OC_TILES, 128, C, 9).transpose(2, 0, 3, 1)
    w2 = np.zeros((C, WTOT), f8)
    w2[:, 0 : 2 * WHI] = whi_t.reshape(C, 2 * WHI)
    w2[:, 2 * WHI : 2 * WHI + WLO] = wlo_t[:, 0, :7, :].reshape(C, WLO)
    w2[:, 2 * WHI + WLO : WTOT] = wlo_t[:, 1, :7, :].reshape(C, WLO)

    b2 = np.ascontiguousarray(bias.reshape(OC_TILES, 128).T)

    if "nc" not in _CACHE:
        _CACHE["nc"] = _build()
    nc = _CACHE["nc"]

    in_maps = [
        {"xp": xp[i * NPC : (i + 1) * NPC], "w2": w2, "b2": b2}
        for i in range(N_CORES)
    ]
    res = run_bass_kernel_spmd(nc, in_maps, core_ids=list(range(N_CORES)))
    LAST_RESULTS = res
    return np.concatenate(
        [np.asarray(r["out"]).astype(np.float32) for r in res.results], axis=0
    )

